# revision 42
# baseline (speedup 1.0000x reference)
"""NeuralGraphPool kernel for Trainium2 (8 NeuronCores, data-parallel over batch).

Computation (per molecule b):
    out[a, f] = max(atoms[a, f], max_{d: edges[a,d]>=0} atoms[edges[a,d], f])
                * (any edge valid ? 1 : 0)

Hybrid strategy: the two gather engines run in parallel on disjoint
molecule sets, sized so every hardware unit is near-busy:

  - DMA path (first NDMA molecules, in pairs): SWDGE dma_gather pulls
    self+8 neighbour rows per atom as fp16 (9x HBM amplification, but the
    bus is otherwise idle); the 9-way max tree runs in cheap fp16 2x mode
    on DVE. Degree-0 atoms gather a zero row, so no mask pass is needed.
  - PE path (remaining molecules, in pairs): fp8 DoubleRow one-hot
    matmuls (P^T@hi + P^T@lo at 0.5 cycles/row, stride-0 broadcast
    weights) gather neighbour slots into f32 PSUM. Real-HW constraints:
    GPSIMD cannot touch PSUM and an instruction may read at most ONE PSUM
    operand, so slot rounds either feed a DVE max-chain seeded by the
    ScalarE-masked self row, or are copied to SBUF fp16 by ScalarE
    ('a' rounds) and pair-merged on DVE.

fp16 result DMAs out; host casts to f32.
"""

import numpy as np

import concourse.bacc as bacc
import concourse.mybir as mybir
from concourse.tile import TileContext
from concourse.bass_utils import run_bass_kernel_spmd

# Problem constants (hardcoded per harness contract).
B, A, D, F = 256, 128, 8, 512
N_CORES = 8
BPC = B // N_CORES           # molecules per core (32)
NPAIR = BPC // 2
S = D + 1                    # gather slots per atom (self + 8 neighbours)
NI = 2 * S * A               # gather indices per molecule pair (2304)
IDX_COLS = NI // 16          # idx free-dim per pair (144)
ZR = BPC * A                 # zero-row index in the gather table

# knobs ---------------------------------------------------------------
NDMA = 18         # molecules on the DMA-gather path (even; rest on PE)
USE_DR = True     # fp8 DoubleRow gathers on the PE path
# PE-path per-molecule slot-round patterns, cycled by molecule index:
# 'c' = DVE max-chain absorbs the round's 2 PSUM banks,
# 'a' = ScalarE copies the 2 banks to fp16 leaves (merged later on DVE)
EXIT_PATTERNS = ("aaaa",)
# early PE molecules run full DVE chains: their work fills the DVE idle
# window while the first DMA gathers are still in flight
CHAIN_MOLS = ()

_cached = {}


def _build_kernel():
    if "nc" in _cached:
        return _cached["nc"]
    nc = bacc.Bacc("TRN2", num_devices=N_CORES)
    f16 = mybir.dt.float16
    f32 = mybir.dt.float32
    f8 = mybir.dt.float8e4
    MAX = mybir.AluOpType.max
    DR = mybir.MatmulPerfMode.DoubleRow
    NDMA_PAIR = NDMA // 2

    atomsg = nc.declare_dram_parameter(
        "atomsg", [BPC * A + 16, F], f16, isOutput=False)
    gidx = nc.declare_dram_parameter(
        "gidx", [128, max(NDMA_PAIR, 1) * IDX_COLS], mybir.dt.int16,
        isOutput=False)
    atomspk = nc.declare_dram_parameter(
        "atomspk", [A, BPC * 2 * F], f8, isOutput=False)
    onehot = nc.declare_dram_parameter(
        "onehot", [128, BPC * D * 128], f8, isOutput=False)
    maskt = nc.declare_dram_parameter("maskt", [128, BPC], f32, isOutput=False)
    out = nc.declare_dram_parameter("out", [A, BPC * F], f16, isOutput=True)

    with TileContext(nc) as tc:
        with (
            tc.tile_pool(name="const", bufs=1) as cpool,
            tc.tile_pool(name="g", bufs=3) as gpool,
            tc.tile_pool(name="tmp", bufs=3) as tpool,
            tc.tile_pool(name="pk", bufs=3) as apool,
            tc.tile_pool(name="oh", bufs=3) as ohpool,
            tc.tile_pool(name="ps", bufs=1, space="PSUM") as pspool,
            tc.tile_pool(name="leaf", bufs=3) as lpool,
            tc.tile_pool(name="mid", bufs=3) as mpool,
            tc.tile_pool(name="outp", bufs=4) as opool,
        ):
            mask_all = cpool.tile([128, BPC], f32)
            idx_all = cpool.tile(
                [128, max(NDMA_PAIR, 1) * IDX_COLS], mybir.dt.int16)

            # pair-0's indices load first so the first gather starts early;
            # the rest follow while gathers serialize on Pool anyway
            nc.sync.dma_start(out=idx_all[:, :IDX_COLS],
                              in_=gidx[:, :IDX_COLS])
            nc.sync.dma_start(out=mask_all[:], in_=maskt[:])
            if NDMA_PAIR > 1:
                nc.sync.dma_start(out=idx_all[:, IDX_COLS:],
                                  in_=gidx[:, IDX_COLS:])

            def dma_pair(q):
                """Molecules 2q, 2q+1 via SWDGE gather + fp16 DVE tree.

                NOTE: prepare_only+trigger_dma sims 0.15us faster and
                frees Pool in the cost model, but produces corrupt gathers
                on real hardware (rel err 62!) - keep the fused form.
                """
                mA = 2 * q
                g = gpool.tile([A, 2 * S, F], f16)
                nc.gpsimd.dma_gather(
                    out_ap=g[:],
                    in_ap=atomsg[:],
                    idxs_ap=idx_all[:, q * IDX_COLS:(q + 1) * IDX_COLS],
                    num_idxs=NI,
                    num_idxs_reg=NI,
                    elem_size=F,
                    single_packet=False,
                )
                gv = g[:].rearrange("p (j s) f -> p j s f", s=S)
                t = tpool.tile([A, 2, 4, F], f16)
                nc.vector.tensor_tensor(
                    out=t[:], in0=gv[:, :, 1:9:2, :], in1=gv[:, :, 2:9:2, :],
                    op=MAX)
                u = tpool.tile([A, 2, 2, F], f16)
                nc.vector.tensor_tensor(
                    out=u[:], in0=t[:, :, 0:2, :], in1=t[:, :, 2:4, :], op=MAX)
                v = tpool.tile([A, 2, F], f16)
                nc.vector.tensor_tensor(
                    out=v[:], in0=u[:, :, 0, :], in1=u[:, :, 1, :], op=MAX)
                w = opool.tile([A, 2, F], f16, name="w")
                nc.vector.tensor_tensor(
                    out=w[:], in0=v[:], in1=gv[:, :, 0, :], op=MAX)
                nc.sync.dma_start(
                    out=out[:, mA * F:(mA + 2) * F].rearrange(
                        "p (m f) -> p m f", m=2),
                    in_=w[:])

            def pe_pair(p, drain):
                """Molecules 2p, 2p+1 via fp8 DoubleRow one-hot matmuls."""
                mA = 2 * p
                pk = apool.tile([128, 2, 2, F], f8)
                nc.sync.dma_start(
                    out=pk[:],
                    in_=atomspk[:, mA * 2 * F:(mA + 2) * 2 * F].rearrange(
                        "p (m j f) -> p m j f", m=2, j=2))
                oh = ohpool.tile([128, 2, D, 128], f8)
                nc.sync.dma_start(
                    out=oh[:],
                    in_=onehot[:, mA * D * 128:(mA + 2) * D * 128].rearrange(
                        "p (m d a) -> p m d a", m=2, d=D))

                def mm(dst, w2, mol):
                    if USE_DR:
                        nc.tensor.matmul(
                            out=dst, lhsT=w2, rhs=pk[:, mol, :, :],
                            start=True, stop=True, perf_mode=DR)
                    else:
                        nc.tensor.matmul(
                            out=dst, lhsT=w2[:, 0, :], rhs=pk[:, mol, 0, :],
                            start=True, stop=False)
                        nc.tensor.matmul(
                            out=dst, lhsT=w2[:, 1, :], rhs=pk[:, mol, 1, :],
                            start=False, stop=True)

                h = opool.tile([128, 2, F], f16, name="h")
                for mol in range(2):
                    mi = mA + mol
                    s16 = mpool.tile([128, F], f16, name="s16")
                    nc.scalar.activation(
                        out=s16[:], in_=pk[:, mol, 0, :],
                        func=mybir.ActivationFunctionType.Copy,
                        bias=0.0, scale=mask_all[:, mi:mi + 1])
                    pat = ("cccc" if mi in CHAIN_MOLS
                           else EXIT_PATTERNS[mi % len(EXIT_PATTERNS)])
                    lv = (lpool.tile([128, 8, F], f16, name=f"lv{mol}")
                          if "a" in pat else None)
                    chain = s16
                    napair = 0
                    for r in range(4):
                        ps = pspool.tile([128, 2, F], f32,
                                         name=f"ps{(4 * mol + r) % 4}")
                        for k in range(2):
                            w2 = (oh[:, mol, 2 * r + k, :].unsqueeze(1)
                                  .broadcast_to([128, 2, 128]))
                            mm(ps[:, k, :], w2, mol)
                        if pat[r] == "c":
                            vv = mpool.tile([128, F], f16, name="va")
                            nc.vector.tensor_tensor(
                                out=vv[:], in0=ps[:, 0, :], in1=chain[:],
                                op=MAX)
                            if pat == "cccc" and r == 3:
                                # final chain step writes the output row
                                nc.vector.tensor_tensor(
                                    out=h[:, mol, :], in0=ps[:, 1, :],
                                    in1=vv[:], op=MAX)
                                chain = None
                            else:
                                vv2 = mpool.tile([128, F], f16, name="vb")
                                nc.vector.tensor_tensor(
                                    out=vv2[:], in0=ps[:, 1, :], in1=vv[:],
                                    op=MAX)
                                chain = vv2
                        else:
                            nc.scalar.activation(
                                out=lv[:, 2 * napair:2 * napair + 2, :],
                                in_=ps[:],
                                func=mybir.ActivationFunctionType.Copy,
                                bias=0.0, scale=1.0)
                            napair += 1

                    if chain is None:
                        continue  # cccc: h already written by the chain
                    # DVE merges of the Act leaves (fp16 2x), widest first
                    nl = 2 * napair
                    m1 = mpool.tile([128, 4, F], f16, name="m1")
                    nc.vector.tensor_tensor(
                        out=m1[:, :napair, :], in0=lv[:, 0:nl:2, :],
                        in1=lv[:, 1:nl:2, :], op=MAX)
                    if napair == 4:
                        m2 = mpool.tile([128, 2, F], f16, name="m2")
                        nc.vector.tensor_tensor(
                            out=m2[:], in0=m1[:, 0:4:2, :],
                            in1=m1[:, 1:4:2, :], op=MAX)
                        m3 = mpool.tile([128, F], f16, name="m3")
                        nc.vector.tensor_tensor(
                            out=m3[:], in0=m2[:, 0, :], in1=m2[:, 1, :],
                            op=MAX)
                    elif napair == 3:
                        m2 = mpool.tile([128, F], f16, name="m2")
                        nc.vector.tensor_tensor(
                            out=m2[:], in0=m1[:, 0, :], in1=m1[:, 1, :],
                            op=MAX)
                        m3 = mpool.tile([128, F], f16, name="m3")
                        nc.vector.tensor_tensor(
                            out=m3[:], in0=m2[:], in1=m1[:, 2, :], op=MAX)
                    else:
                        m3 = mpool.tile([128, F], f16, name="m3")
                        nc.vector.tensor_tensor(
                            out=m3[:], in0=m1[:, 0, :], in1=m1[:, 1, :],
                            op=MAX)
                    nc.vector.tensor_tensor(
                        out=h[:, mol, :], in0=m3[:], in1=chain[:], op=MAX)

                nc.sync.dma_start(
                    out=out[:, mA * F:(mA + 2) * F].rearrange(
                        "p (m f) -> p m f", m=2),
                    in_=h[:])

            # interleave the two paths so all engines stay busy
            NDP = NDMA_PAIR
            pe_ps = list(range(NDP, NPAIR))
            dma_qs = list(range(NDP))
            sched = []
            i = j = 0
            while i < len(dma_qs) or j < len(pe_ps):
                if i < len(dma_qs):
                    sched.append(("dma", dma_qs[i])); i += 1
                if j < len(pe_ps):
                    sched.append(("pe", pe_ps[j])); j += 1
            for kind, idx in sched:
                if kind == "dma":
                    dma_pair(idx)
                else:
                    pe_pair(idx, idx == pe_ps[-1])
    nc.compile()
    _cached["nc"] = nc
    return nc


def _host_prep(atoms, bonds, edges):
    """Build per-core input maps. atoms (B,A,F) f32; edges (B,A,D) int32."""
    del bonds  # unused by the layer
    f8np = mybir.dt.np(mybir.dt.float8e4)
    NDMA_PAIR = NDMA // 2
    a_idx = np.arange(A, dtype=np.int64)[None, :, None]            # (1,A,1)
    e = edges.astype(np.int64)
    valid = e >= 0
    e_fixed = np.where(valid, e, a_idx)                            # (B,A,D)
    mask = valid.any(axis=2).astype(np.float32)                    # (B,A)
    atoms16_full = atoms.astype(np.float16)                        # (B,A,F)
    hi = atoms.astype(f8np)                                        # (B,A,F) fp8
    lo = (atoms - hi.astype(np.float32)).astype(f8np)
    iota = np.arange(128, dtype=np.int64)

    in_maps = []
    for c in range(N_CORES):
        mol = slice(c * BPC, (c + 1) * BPC)
        # gather table (BPC*A+16, F) fp16 with zero rows at the end
        at16 = np.zeros((BPC * A + 16, F), np.float16)
        at16[:BPC * A] = atoms16_full[mol].reshape(BPC * A, F)
        # gather indices for the first NDMA molecules, v1 dma_gather layout
        base = (np.arange(BPC, dtype=np.int64) * A)[:, None, None]  # (BPC,1,1)
        slots = np.concatenate(
            [np.broadcast_to(a_idx, (BPC, A, 1)), e_fixed[mol]], axis=2)
        flat = slots + base                                        # (BPC,A,S)
        # degree-0 atoms read the zero row for every slot (mask folds away)
        flat = np.where(mask[mol][:, :, None] > 0, flat, ZR)
        per_pair = flat[:NDMA].astype(np.int16).transpose(0, 2, 1).reshape(
            max(NDMA_PAIR, 1), NI)                    # i = j*S*A + s*A + a
        idx_lay = per_pair.reshape(max(NDMA_PAIR, 1), IDX_COLS, 16
                                   ).transpose(0, 2, 1)
        idx16 = np.ascontiguousarray(
            np.tile(idx_lay, (1, 8, 1)).transpose(1, 0, 2).reshape(
                128, max(NDMA_PAIR, 1) * IDX_COLS))
        # PE-path tables
        pk = np.stack([hi[mol], lo[mol]], axis=2)                  # (BPC,A,2,F)
        pk = np.ascontiguousarray(
            pk.transpose(1, 0, 2, 3).reshape(A, BPC * 2 * F))
        ohb = ((e_fixed[mol][:, :, :, None] == iota)
               & (mask[mol][:, :, None, None] > 0))               # (BPC,A,D,128)
        oh = np.ascontiguousarray(
            ohb.transpose(3, 0, 2, 1).reshape(128, BPC * D * 128)).astype(f8np)
        mk = np.ascontiguousarray(mask[mol].T)                     # (A=128, BPC)
        in_maps.append({"atomsg": at16, "gidx": idx16, "atomspk": pk,
                        "onehot": oh, "maskt": mk})
    return in_maps


def kernel(atoms, bonds, edges, _want_timing=False, **_ignored):
    nc = _build_kernel()
    in_maps = _host_prep(np.asarray(atoms, dtype=np.float32), bonds,
                         np.asarray(edges, dtype=np.int32))
    res = run_bass_kernel_spmd(nc, in_maps, list(range(N_CORES)),
                               trace=False)
    outs = [
        res.results[c]["out"].reshape(A, BPC, F).transpose(1, 0, 2)
        for c in range(N_CORES)
    ]
    full = np.concatenate(outs, axis=0).astype(np.float32)
    if _want_timing:
        return full, res
    return full
